# revision 3
# baseline (speedup 1.0000x reference)
"""BitLinear (LayerNorm + 8-bit act quant + ternary weight quant + GEMM) on 8 TRN2 cores.

Sharding: data-parallel over flattened rows (B*S = 8192 -> 1024 rows/core).
Each core holds the full quantized transposed weight (bf16, exact for ternary
values) and computes LN + activation quantization for its own rows only, so
there is no redundant vector work and no collective.

Weight quantization (absmean ternary) is precomputed on the host: the weight
scale gamma is a single global scalar and the quantized weights are static --
the standard BitLinear inference setup (the sharding hint explicitly allows
precomputing the weight scale).
"""

import math

import numpy as np
import ml_dtypes

import concourse.bass as bass
import concourse.bacc as bacc
import concourse.mybir as mybir
import concourse.tile as tile
from concourse.bass_utils import run_bass_kernel_spmd
from concourse.masks import make_identity

# Problem shapes (hardcoded per contract -- kernel.py must be self-contained).
B, S, K, N = 2, 4096, 2048, 8192
M_TOTAL = B * S              # 8192 flattened rows
N_CORES = 8
M_LOC = M_TOTAL // N_CORES   # 1024 rows per core
P = 128                      # partitions
M_TILES = M_LOC // P         # 8
K_TILES = K // P             # 16
N_MM = 512                   # moving-operand free dim per matmul (1 PSUM bank)
N_CHUNK = 1024               # weight-stream chunk along N (2KiB DMA lines)
N_CHUNKS = N // N_CHUNK      # 8

EPS_LN = 1e-5
EPS_Q = 1e-5
MAGIC = 12582912.0           # 1.5 * 2**23: fp32 add/sub performs round-to-nearest-even

FP32 = mybir.dt.float32
BF16 = mybir.dt.bfloat16


def _build_nc():
    nc = bacc.Bacc("TRN2", target_bir_lowering=False, debug=False, num_devices=N_CORES)

    xc_d = nc.dram_tensor("xc", [M_LOC, K], FP32, kind="ExternalInput")
    wt_d = nc.dram_tensor("wt", [K, N], BF16, kind="ExternalInput")
    gs_d = nc.dram_tensor("gs", [P, 1], FP32, kind="ExternalInput")   # gamma/127 replicated
    bs_d = nc.dram_tensor("bs", [N], FP32, kind="ExternalInput")
    out_d = nc.dram_tensor("out", [M_LOC, N], FP32, kind="ExternalOutput")

    with tile.TileContext(nc) as tc:
        with (
            tc.tile_pool(name="singles", bufs=1) as singles,
            tc.tile_pool(name="xin", bufs=2) as xin_pool,
            tc.tile_pool(name="xn", bufs=2) as xn_pool,
            tc.tile_pool(name="stats", bufs=4) as stats_pool,
            tc.tile_pool(name="xqt", bufs=1) as xqt_pool,
            tc.tile_pool(name="wstream", bufs=2) as w_pool,
            tc.tile_pool(name="osb", bufs=4) as o_pool,
            tc.tile_pool(name="pst", bufs=2, space="PSUM") as pst_pool,
            tc.tile_pool(name="psg", bufs=4, space="PSUM") as psg_pool,
        ):
            # --- constants ---
            ident = singles.tile([P, P], FP32)
            make_identity(nc, ident)
            gs_t = singles.tile([P, 1], FP32)
            nc.sync.dma_start(gs_t[:], gs_d[:])
            bias_t = singles.tile([P, N], FP32)
            bias_bcast = bass.AP(
                tensor=bs_d.ap().tensor, offset=0, ap=[[0, P]] + bs_d.ap().ap
            )
            nc.sync.dma_start(bias_t[:], bias_bcast)
            rs_all = singles.tile([P, M_TILES], FP32)   # per-row output scale
            eps_t = singles.tile([P, 1], FP32)
            nc.vector.memset(eps_t[:], EPS_LN)
            nmagic_t = singles.tile([P, 1], FP32)
            nc.vector.memset(nmagic_t[:], -MAGIC)
            # x_q^T, bf16, laid out [p=k_inner, k_tile, m_tile, m_inner]
            xqt = xqt_pool.tile([P, K_TILES, M_TILES, P], BF16)

            # --- phase A: LayerNorm + activation quant + transpose, per row-tile ---
            for m in range(M_TILES):
                xt = xin_pool.tile([P, K], FP32)
                nc.sync.dma_start(xt[:], xc_d[m * P : (m + 1) * P, :])

                st = stats_pool.tile([P, K // 512, 6], FP32)
                for i in range(K // 512):
                    nc.vector.bn_stats(st[:, i, :], xt[:, i * 512 : (i + 1) * 512])
                mv = stats_pool.tile([P, 2], FP32)
                nc.vector.bn_aggr(mv[:], st[:])

                # rstd = 1/sqrt(var + eps)
                std = stats_pool.tile([P, 1], FP32)
                nc.scalar.activation(
                    std[:], mv[:, 1:2], mybir.ActivationFunctionType.Sqrt, bias=eps_t[:]
                )
                rstd = stats_pool.tile([P, 1], FP32)
                nc.vector.reciprocal(rstd[:], std[:])

                # xn = (x - mu) * rstd
                xn = xn_pool.tile([P, K], FP32)
                nc.vector.tensor_scalar(
                    out=xn[:],
                    in0=xt[:],
                    scalar1=mv[:, 0:1],
                    scalar2=rstd[:],
                    op0=mybir.AluOpType.subtract,
                    op1=mybir.AluOpType.mult,
                )

                # eta = max(absmax(xn), EPS_Q); inv = 127/eta; rs = eta * gamma/127
                eta = stats_pool.tile([P, 1], FP32)
                nc.vector.tensor_reduce(
                    out=eta[:],
                    in_=xn[:],
                    axis=mybir.AxisListType.X,
                    op=mybir.AluOpType.max,
                    apply_absolute_value=True,
                )
                eta2 = stats_pool.tile([P, 1], FP32)
                nc.vector.tensor_scalar_max(out=eta2[:], in0=eta[:], scalar1=EPS_Q)
                inv = stats_pool.tile([P, 1], FP32)
                nc.vector.reciprocal(inv[:], eta2[:])
                inv127 = stats_pool.tile([P, 1], FP32)
                nc.vector.tensor_scalar_mul(out=inv127[:], in0=inv[:], scalar1=127.0)
                nc.vector.tensor_mul(rs_all[:, m : m + 1], eta2[:], gs_t[:])

                # quantize in place: xn <- xn*inv127 + MAGIC  (int + MAGIC, exact fp32)
                nc.vector.tensor_scalar(
                    out=xn[:],
                    in0=xn[:],
                    scalar1=inv127[:],
                    scalar2=MAGIC,
                    op0=mybir.AluOpType.mult,
                    op1=mybir.AluOpType.add,
                )

                # transpose each [128,128] block on PE; subtract MAGIC on the
                # PSUM->SBUF copy and downcast to bf16 (ints <= 127: exact)
                for kt in range(K_TILES):
                    ps = pst_pool.tile([P, P], FP32)
                    nc.tensor.transpose(ps[:], xn[:, kt * P : (kt + 1) * P], ident[:])
                    nc.scalar.activation(
                        xqt[:, kt, m, :],
                        ps[:],
                        mybir.ActivationFunctionType.Identity,
                        bias=nmagic_t[:],
                    )

            # --- phase B: GEMM out[m,n] = sum_k xq[m,k] wq[k,n], epilogue scale+bias ---
            for nch in range(N_CHUNKS):
                wtile = w_pool.tile([P, K_TILES, N_CHUNK], BF16)
                wt_view = wt_d.ap().rearrange("(kt p) n -> p kt n", p=P)
                nc.sync.dma_start(
                    wtile[:], wt_view[:, :, nch * N_CHUNK : (nch + 1) * N_CHUNK]
                )
                for nj in range(N_CHUNK // N_MM):
                    n0 = nch * N_CHUNK + nj * N_MM
                    for m in range(M_TILES):
                        pt = psg_pool.tile([P, N_MM], FP32)
                        for kt in range(K_TILES):
                            nc.tensor.matmul(
                                pt[:],
                                xqt[:, kt, m, :],
                                wtile[:, kt, nj * N_MM : (nj + 1) * N_MM],
                                start=(kt == 0),
                                stop=(kt == K_TILES - 1),
                            )
                        osb = o_pool.tile([P, N_MM], FP32)
                        nc.scalar.activation(
                            osb[:],
                            pt[:],
                            mybir.ActivationFunctionType.Copy,
                            scale=rs_all[:, m : m + 1],
                        )
                        nc.vector.tensor_add(
                            osb[:], osb[:], bias_t[:, n0 : n0 + N_MM]
                        )
                        nc.sync.dma_start(
                            out_d[m * P : (m + 1) * P, n0 : n0 + N_MM], osb[:]
                        )

    nc.compile()
    return nc


_NC_CACHE = None


def _get_nc():
    global _NC_CACHE
    if _NC_CACHE is None:
        _NC_CACHE = _build_nc()
    return _NC_CACHE


def _weight_gamma(weight: np.ndarray) -> np.float32:
    """absmean scale, matching jnp.maximum(jnp.mean(jnp.abs(w)), EPS_Q) bitwise
    where possible (jax-cpu), falling back to float64 numpy."""
    try:
        import jax

        cpu = jax.devices("cpu")[0]
        with jax.default_device(cpu):
            import jax.numpy as jnp

            g = jnp.maximum(jnp.mean(jnp.abs(jnp.asarray(weight))), EPS_Q)
            return np.float32(np.asarray(g))
    except Exception:
        return np.float32(max(np.mean(np.abs(weight), dtype=np.float64), EPS_Q))


def kernel(x: np.ndarray, weight: np.ndarray, bias: np.ndarray) -> np.ndarray:
    assert x.shape == (B, S, K) and weight.shape == (N, K) and bias.shape == (N,)

    # host-side weight quantization (static in real BitLinear inference)
    gamma = _weight_gamma(weight)
    w_q = np.round(np.clip(weight.astype(np.float32) / gamma, -1.0, 1.0))
    wt_bf16 = np.ascontiguousarray(w_q.T).astype(ml_dtypes.bfloat16)  # [K, N]

    gs = np.full((P, 1), gamma / np.float32(127.0), dtype=np.float32)
    bias_f = np.ascontiguousarray(bias.astype(np.float32))
    x_flat = np.ascontiguousarray(x.reshape(M_TOTAL, K).astype(np.float32))

    nc = _get_nc()
    in_maps = [
        {
            "xc": x_flat[c * M_LOC : (c + 1) * M_LOC],
            "wt": wt_bf16,
            "gs": gs,
            "bs": bias_f,
        }
        for c in range(N_CORES)
    ]
    res = run_bass_kernel_spmd(nc, in_maps, list(range(N_CORES)))
    out = np.concatenate([res.results[c]["out"] for c in range(N_CORES)], axis=0)
    return out.reshape(B, S, N).astype(np.float32, copy=False)
